# revision 1
# baseline (speedup 1.0000x reference)
"""Trainium2 Bass kernel for nn_Attention_85710367359111.

Full (unsharded) numpy inputs in, full output out. Internally:
tensor-parallel over heads (16 heads / 8 cores = 2 heads per core),
on-device AllToAll to re-shard from head-parallel to token-parallel
before the output projection, host-side concat of the 8 token blocks.

Per-core pipeline (all feature-major [feat_on_partitions, tokens]):
  A  qkv^T = w_qkv_slice @ x^T           (PE, fp32r)
  B  RMS stats via selector-matmul -> sqrt(ACT) -> reciprocal(DVE),
     scale+norm_w via rank-2 broadcast matmul, RoPE via +-1 permutation
     matmul + cos/sin elementwise (DVE)
  B' v^T -> v token-major via PE transposes (with appended ones column)
  C  scores^T = k^T.T @ q^T per (batch,head) -> exp (ACT) ->
     o_aug^T = [v|1].T @ exp(scores^T)  (PSUM-accumulated over k tiles),
     softmax denominator = last row of o_aug; normalize via
     reciprocal + rank-1 broadcast matmul + multiply
  D  DMA o to DRAM laid out [tok_block, d_local, tok_in], AllToAll,
     out^T = w_out^T.T @ gathered  (row-parallel, no all-reduce needed)
"""

import math
import os

import numpy as np

import concourse.bacc as bacc
import concourse.bass as bass
import concourse.tile as tile
from concourse import mybir
from concourse.bass_utils import run_bass_kernel_spmd

# ---------------------------------------------------------------- config

B, S, D, H, E = 2, 2048, 1024, 16, 64
NC = 8                      # cores
HPC = H // NC               # heads per core = 2
DL = HPC * E                # local d slice = 128
T = B * S                   # total tokens
TB = T // NC                # tokens per A2A block
KT = S // 128               # k tiles per batch
NTT = T // 128              # total tok tiles
QC = min(512, S)            # q chunk
NQC = S // QC               # q chunks per batch
EPS = float(np.finfo(np.float32).eps)

MM_DT = mybir.dt.float32r if os.environ.get("KMM_DT", "f32r") == "f32r" else mybir.dt.float32


def _r(ap):
    return ap


def build_nc(s=S, reps=1):
    global S, T, TB, KT, NTT, QC, NQC
    S = s
    T = B * S
    TB = T // NC
    KT = S // 128
    NTT = T // 128
    QC = min(512, S)
    NQC = S // QC

    f32 = mybir.dt.float32
    nc = bacc.Bacc("TRN2", target_bir_lowering=False, debug=False, num_devices=NC)

    # ------------- DRAM I/O
    rdt = MM_DT
    xT_d = nc.dram_tensor("xT", [D, T], rdt, kind="ExternalInput")
    wq_d = nc.dram_tensor("wqT", [D, DL], rdt, kind="ExternalInput")
    wk_d = nc.dram_tensor("wkT", [D, DL], rdt, kind="ExternalInput")
    wv_d = nc.dram_tensor("wvT", [D, DL], rdt, kind="ExternalInput")
    wo_d = nc.dram_tensor("woT", [D, D], rdt, kind="ExternalInput")
    cos_d = nc.dram_tensor("cosT", [128, T], f32, kind="ExternalInput")
    sin_d = nc.dram_tensor("sinT", [128, T], f32, kind="ExternalInput")
    sel_d = nc.dram_tensor("sel", [128, 2], f32, kind="ExternalInput")
    wsel_d = nc.dram_tensor("wsel", [2, 128], f32, kind="ExternalInput")
    perm_d = nc.dram_tensor("permT", [128, 128], f32, kind="ExternalInput")
    iden_d = nc.dram_tensor("iden", [128, 128], f32, kind="ExternalInput")
    ones_d = nc.dram_tensor("ones1", [1, E], f32, kind="ExternalInput")
    epsb_d = nc.dram_tensor("epsb", [2, 1], f32, kind="ExternalInput")
    onec_d = nc.dram_tensor("onecol", [128, NTT], rdt, kind="ExternalInput")

    osh_d = nc.dram_tensor("o_shard", [NC * DL, TB], rdt)
    oga_d = nc.dram_tensor("o_gath", [NC * DL, TB], rdt)
    out_d = nc.dram_tensor("out_t", [D, TB], f32, kind="ExternalOutput")
    DBG = bool(os.environ.get("KDEBUG"))
    PH = os.environ.get("KPHASES", "ABCD")
    if DBG:
        dbg = {}
        for nm, shp, dt_ in [("dbg_q01", [128, T], f32), ("dbg_k01", [128, T], f32),
                             ("dbg_v01", [128, T], f32), ("dbg_qhat", [128, T], rdt),
                             ("dbg_khat", [128, T], rdt), ("dbg_vtok", [128, NTT * 130], rdt),
                             ("dbg_osh", [NC * DL, TB], rdt), ("dbg_oga", [NC * DL, TB], rdt),
                             ("dbg_ex", [128, 2 * QC], rdt), ("dbg_osb", [65, QC], f32)]:
            dbg[nm] = nc.dram_tensor(nm, shp, dt_, kind="ExternalOutput")

    xT_v = xT_d.ap().rearrange("(dt p) t -> dt p t", p=128)     # [8,128,T]
    wo_v = wo_d.ap().rearrange("(dt p) o -> dt p o", p=128)     # [8,128,D]
    oga_v = oga_d.ap().rearrange("(dt p) t -> dt p t", p=128)   # [8,128,TB]

    from contextlib import ExitStack

    with tile.TileContext(nc) as tc, ExitStack() as ctx:
      for _rep in range(reps):
          with ExitStack() as ctx2:
            # persistent (whole-kernel) buffers
            pers = ctx2.enter_context(tc.tile_pool(name="pers", bufs=1))
            qhat = pers.tile([128, T], rdt, tag="qhat")     # [q_h0; q_h1] normed+roped
            khat = pers.tile([128, T], rdt, tag="khat")
            vtok = pers.tile([128, NTT, 130], rdt, tag="vtok")  # token-major v + ones cols
            wq_s = pers.tile([128, D], rdt, tag="wq")       # w tiles, [128(dt rows), 8*128]
            wk_s = pers.tile([128, D], rdt, tag="wk")
            wv_s = pers.tile([128, D], rdt, tag="wv")
            sel_s = pers.tile([128, 2], f32, tag="sel")
            wsel_s = pers.tile([2, 128], f32, tag="wsel")
            perm_s = pers.tile([128, 128], f32, tag="perm")
            iden_s = pers.tile([128, 128], f32, tag="iden")
            ones_s = pers.tile([1, E], f32, tag="ones1")
            epsb_s = pers.tile([2, 1], f32, tag="epsb")

            for dt_i in range(8):
                nc.sync.dma_start(wq_s[:, bass.ts(dt_i, 128)], wq_d.ap()[bass.ts(dt_i, 128), :])
                nc.sync.dma_start(wk_s[:, bass.ts(dt_i, 128)], wk_d.ap()[bass.ts(dt_i, 128), :])
                nc.sync.dma_start(wv_s[:, bass.ts(dt_i, 128)], wv_d.ap()[bass.ts(dt_i, 128), :])
            nc.sync.dma_start(sel_s[:], sel_d.ap())
            nc.sync.dma_start(wsel_s[:], wsel_d.ap())
            nc.sync.dma_start(perm_s[:], perm_d.ap())
            nc.sync.dma_start(iden_s[:], iden_d.ap())
            nc.sync.dma_start(ones_s[:], ones_d.ap())
            nc.sync.dma_start(epsb_s[:], epsb_d.ap())

            # ones columns of vtok (col 64 for head0, col 129 for head1)
            nc.sync.dma_start(vtok[:, :, 64], onec_d.ap())
            nc.sync.dma_start(vtok[:, :, 129], onec_d.ap())

            NCH = T // 512  # 512-wide token chunks for phases A/B

            with tc.tile_pool(name="ab", bufs=1) as ab, \
                 tc.tile_pool(name="xt", bufs=12) as xtp, \
                 tc.tile_pool(name="sq", bufs=3) as sqp, \
                 tc.tile_pool(name="cs", bufs=3) as csp, \
                 tc.tile_pool(name="st", bufs=3) as stp, \
                 tc.tile_pool(name="ps", bufs=2, space=bass.MemorySpace.PSUM) as psp, \
                 tc.tile_pool(name="pss", bufs=2, space=bass.MemorySpace.PSUM) as pss, \
                 tc.tile_pool(name="tmp", bufs=3) as tmpp:
                q01 = ab.tile([128, T], f32, tag="q01")
                k01 = ab.tile([128, T], f32, tag="k01")
                v01 = ab.tile([128, T], f32, tag="v01")

                # ---------------- phase A: qkv^T = W @ x^T, feature-major
                for c in range(NCH):
                    cs = bass.ts(c, 512)
                    xts = []
                    for dt_i in range(8):
                        xt = xtp.tile([128, 512], rdt, tag="xt")
                        nc.sync.dma_start(xt[:], xT_v[dt_i, :, cs])
                        xts.append(xt)
                    for w_s, dest in ((wq_s, q01), (wk_s, k01), (wv_s, v01)):
                        ps = psp.tile([128, 512], f32, tag="ps")
                        for dt_i in range(8):
                            nc.tensor.matmul(
                                ps[:], _r(w_s[:, bass.ts(dt_i, 128)]), _r(xts[dt_i][:]),
                                start=(dt_i == 0), stop=(dt_i == 7),
                            )
                        nc.scalar.copy(dest[:, cs], ps[:])

                # ------- phase B (fused): stats -> alpha -> scale -> rope, per chunk
                for c in range(NCH if "B" in PH else 0):
                    cs = bass.ts(c, 512)
                    cos_t = csp.tile([128, 512], f32, tag="cos")
                    sin_t = csp.tile([128, 512], f32, tag="sin")
                    nc.sync.dma_start(cos_t[:], cos_d.ap()[:, cs])
                    nc.sync.dma_start(sin_t[:], sin_d.ap()[:, cs])
                    for src_t, dest in ((q01, qhat), (k01, khat)):
                        sq = sqp.tile([128, 512], f32, tag="sq")
                        nc.scalar.activation(sq[:], src_t[:, cs],
                                             mybir.ActivationFunctionType.Square)
                        st = pss.tile([2, 512], f32, tag="pss")
                        nc.tensor.matmul(st[:], _r(sel_s[:]), _r(sq[:]), start=True, stop=True)
                        sqv = stp.tile([2, 512], f32, tag="sqv")
                        # sqrt(8*mean + 8*eps); reciprocal gives alpha/sqrt(8)
                        nc.scalar.activation(sqv[:], st[:],
                                             mybir.ActivationFunctionType.Sqrt,
                                             bias=epsb_s[:], scale=8.0)
                        alpha = stp.tile([2, 512], f32, tag="alpha")
                        nc.vector.reciprocal_approx_fast(alpha[:], sqv[:])
                        sps = pss.tile([128, 512], f32, tag="pss")
                        nc.tensor.matmul(sps[:], _r(wsel_s[:]), _r(alpha[:]),
                                         start=True, stop=True)
                        qs = tmpp.tile([128, 512], f32, tag="qs")
                        nc.vector.tensor_mul(qs[:], src_t[:, cs], sps[:])
                        yp = psp.tile([128, 512], f32, tag="ps")
                        nc.tensor.matmul(yp[:], _r(perm_s[:]), _r(qs[:]), start=True, stop=True)
                        t1 = tmpp.tile([128, 512], f32, tag="t1")
                        nc.vector.tensor_mul(t1[:], qs[:], cos_t[:])
                        t2 = tmpp.tile([128, 512], f32, tag="t2")
                        nc.vector.tensor_mul(t2[:], yp[:], sin_t[:])
                        nc.vector.tensor_add(dest[:, cs], t1[:], t2[:])

                if DBG:
                    nc.sync.dma_start(dbg["dbg_q01"].ap(), q01[:])
                    nc.sync.dma_start(dbg["dbg_k01"].ap(), k01[:])
                    nc.sync.dma_start(dbg["dbg_v01"].ap(), v01[:])
                    nc.sync.dma_start(dbg["dbg_qhat"].ap(), qhat[:])
                    nc.sync.dma_start(dbg["dbg_khat"].ap(), khat[:])

                # ---------------- phase B': v -> token-major (+ ones)
                for g in range(NTT // 4 if "B" in PH else 0):
                    pt = psp.tile([128, 4, 128], f32, tag="ps")
                    for j in range(4):
                        tt = g * 4 + j
                        nc.tensor.transpose(pt[:, j, :], v01[:, bass.ts(tt, 128)], iden_s[:])
                    nc.vector.tensor_copy(vtok[:, bass.ts(g, 4), 0:64], pt[:, :, 0:64])
                    nc.vector.tensor_copy(vtok[:, bass.ts(g, 4), 65:129], pt[:, :, 64:128])

            # ---------------- phase C: attention per (batch, qchunk)
            wop = ctx2.enter_context(tc.tile_pool(name="wo", bufs=1))
            wo_s = wop.tile([128, 8, D], rdt, tag="wo")
            for dt_i in range(8):
                nc.sync.dma_start(wo_s[:, dt_i, :], wo_v[dt_i])
            with tc.tile_pool(name="scps", bufs=2, space=bass.MemorySpace.PSUM) as scps, \
                 tc.tile_pool(name="ops", bufs=1, space=bass.MemorySpace.PSUM) as ops, \
                 tc.tile_pool(name="bcps", bufs=1, space=bass.MemorySpace.PSUM) as bcps, \
                 tc.tile_pool(name="expp", bufs=4) as expp, \
                 tc.tile_pool(name="osb", bufs=2) as osbp, \
                 tc.tile_pool(name="den", bufs=2) as denp, \
                 tc.tile_pool(name="ofin", bufs=2) as ofinp:
                for b in range(B if "C" in PH else 0):
                    for qc in range(NQC):
                        qs_ = slice(b * S + qc * QC, b * S + (qc + 1) * QC)
                        oa0 = ops.tile([65, QC], f32, tag="oa0")
                        oa1 = ops.tile([65, QC], f32, tag="oa1")
                        for kt in range(KT):
                            ks = slice(b * S + kt * 128, b * S + (kt + 1) * 128)
                            sc = scps.tile([128, 2 * QC], f32, tag="sc")
                            nc.tensor.matmul(sc[:, 0:QC], _r(khat[0:64, ks]),
                                             _r(qhat[0:64, qs_]), start=True, stop=True)
                            nc.tensor.matmul(sc[:, QC:2 * QC], _r(khat[64:128, ks]),
                                             _r(qhat[64:128, qs_]), start=True, stop=True)
                            ex = expp.tile([128, 2 * QC], rdt, tag="ex")
                            nc.scalar.activation(ex[:], sc[:], mybir.ActivationFunctionType.Exp)
                            if DBG and b == 0 and qc == 0 and kt == 0:
                                nc.sync.dma_start(dbg["dbg_ex"].ap(), ex[:])
                            tt = b * KT + kt
                            for h in range(HPC):
                                nc.tensor.matmul(
                                    (oa0 if h == 0 else oa1)[:],
                                    _r(vtok[:, tt, h * 65:h * 65 + 65]),
                                    _r(ex[:, bass.ts(h, QC)]),
                                    start=(kt == 0), stop=(kt == KT - 1),
                                )
                        for h, oa in ((0, oa0), (1, oa1)):
                            osb = osbp.tile([65, QC], f32, tag="osb")
                            nc.vector.tensor_copy(osb[:], oa[:])
                            if DBG and b == 0 and qc == 0 and h == 0:
                                nc.sync.dma_start(dbg["dbg_osb"].ap(), osb[:])
                            den0 = denp.tile([1, QC], f32, tag="den0")
                            nc.sync.dma_start(den0[:], osb[64:65, :])
                            dr = denp.tile([1, QC], f32, tag="dr")
                            nc.vector.reciprocal_approx_fast(dr[:], den0[:])
                            bc = bcps.tile([64, QC], f32, tag="bc")
                            nc.tensor.matmul(bc[:], _r(ones_s[:]), _r(dr[:]),
                                             start=True, stop=True)
                            of = ofinp.tile([64, QC], rdt, tag="of")
                            nc.vector.tensor_mul(of[:], osb[0:64, :], bc[:])
                            # o_shard row = tok_block*DL + h*64
                            tok0 = b * S + qc * QC
                            if QC <= TB:
                                blk = tok0 // TB
                                off = tok0 % TB
                                nc.sync.dma_start(
                                    osh_d.ap()[blk * DL + h * 64: blk * DL + h * 64 + 64,
                                               off:off + QC], of[:])
                            else:
                                for sb in range(QC // TB):
                                    blk = (tok0 + sb * TB) // TB
                                    nc.sync.dma_start(
                                        osh_d.ap()[blk * DL + h * 64: blk * DL + h * 64 + 64, :],
                                        of[:, bass.ts(sb, TB)])

                # ---------------- phase D: AllToAll + out projection
                if DBG:
                    nc.sync.dma_start(dbg["dbg_vtok"].ap(),
                                      vtok[:].rearrange("p a b -> p (a b)"))
                    nc.sync.dma_start(dbg["dbg_osh"].ap(), osh_d.ap())
                if not os.environ.get("KNO_CC"):
                    nc.gpsimd.collective_compute(
                        "AllToAll", mybir.AluOpType.bypass,
                        replica_groups=[list(range(NC))],
                        ins=[osh_d.ap()], outs=[oga_d.ap()],
                    )

            with tc.tile_pool(name="gd", bufs=1) as gdp, \
                 tc.tile_pool(name="pso", bufs=2, space=bass.MemorySpace.PSUM) as psop, \
                 tc.tile_pool(name="osb2", bufs=2) as osb2p:
                g_s = gdp.tile([128, 8, TB], rdt, tag="g")
                ga_v = osh_d.ap().rearrange("(dt p) t -> dt p t", p=128) \
                    if os.environ.get("KNO_CC") else oga_v
                for dt_i in range(8):
                    nc.sync.dma_start(g_s[:, dt_i, :], ga_v[dt_i])
                if DBG and not os.environ.get("KNO_CC"):
                    nc.sync.dma_start(dbg["dbg_oga"].ap(), oga_d.ap())
                for do in range(8 if "D" in PH else 0):
                    po = psop.tile([128, TB], f32, tag="pso")
                    for dt_i in range(8):
                        nc.tensor.matmul(po[:], _r(wo_s[:, dt_i, bass.ts(do, 128)]),
                                         _r(g_s[:, dt_i, :]),
                                         start=(dt_i == 0), stop=(dt_i == 7))
                    ob = osb2p.tile([128, TB], f32, tag="ob")
                    nc.scalar.copy(ob[:], po[:])
                    nc.sync.dma_start(out_d.ap()[bass.ts(do, 128), :], ob[:])

    nc.compile()
    return nc


def make_inputs(x, position, w_qkv, w_out, norm_w, s=None):
    """Build the 8 per-core input dicts from full inputs."""
    s = s or x.shape[1]
    t = x.shape[0] * s
    xT = np.ascontiguousarray(x.reshape(t, D).T).astype(np.float32)
    cos = position[0]   # [s, E]
    sin = position[1]
    cosT1 = np.ascontiguousarray(cos.T)          # [E, s]
    sinT1 = np.ascontiguousarray(sin.T)
    cosT = np.tile(np.concatenate([cosT1, cosT1], 0), (1, x.shape[0]))  # [128, t]
    sinT = np.tile(np.concatenate([sinT1, sinT1], 0), (1, x.shape[0]))

    sel = np.zeros((128, 2), np.float32)
    sel[0:64, 0] = 1.0 / 64.0
    sel[64:128, 1] = 1.0 / 64.0
    wsel = np.zeros((2, 128), np.float32)
    wsel[0, 0:64] = norm_w
    wsel[1, 64:128] = norm_w
    # rope: y = P t ;  y[i] = -t[2i+1] (i<32), y[32+i] = t[2i]
    P = np.zeros((64, 64), np.float32)
    for i in range(32):
        P[i, 2 * i + 1] = -1.0
        P[32 + i, 2 * i] = 1.0
    Pb = np.zeros((128, 128), np.float32)
    Pb[0:64, 0:64] = P
    Pb[64:128, 64:128] = P
    permT = np.ascontiguousarray(Pb.T)
    iden = np.eye(128, dtype=np.float32)
    ones1 = np.ones((1, E), np.float32)
    woT = np.ascontiguousarray(w_out.T).astype(np.float32)

    w3 = w_qkv.reshape(H, 3, E, D)
    in_maps = []
    for c in range(NC):
        h0, h1 = HPC * c, HPC * c + 1
        wqT = np.ascontiguousarray(
            np.concatenate([w3[h0, 0], w3[h1, 0]], 0).T).astype(np.float32)
        wkT = np.ascontiguousarray(
            np.concatenate([w3[h0, 1], w3[h1, 1]], 0).T).astype(np.float32)
        wvT = np.ascontiguousarray(
            np.concatenate([w3[h0, 2], w3[h1, 2]], 0).T).astype(np.float32)
        in_maps.append({
            "xT": xT, "wqT": wqT, "wkT": wkT, "wvT": wvT, "woT": woT,
            "cosT": cosT.astype(np.float32), "sinT": sinT.astype(np.float32),
            "sel": sel, "wsel": wsel, "permT": permT, "iden": iden, "ones1": ones1,
            "epsb": np.full((2, 1), 8.0 * EPS, np.float32),
            "onecol": np.ones((128, t // 128), np.float32),
        })
    return in_maps


def assemble(results, s=None):
    s = s or S
    t = B * s
    tb = t // NC
    out = np.empty((t, D), np.float32)
    for c in range(NC):
        out[c * tb:(c + 1) * tb, :] = results[c]["out_t"].T
    return out.reshape(B, s, D)


_NC_CACHE = {}


def kernel(x, position, w_qkv, w_out, norm_w, heads):
    x = np.asarray(x, np.float32)
    position = np.asarray(position, np.float32)
    w_qkv = np.asarray(w_qkv, np.float32)
    w_out = np.asarray(w_out, np.float32)
    norm_w = np.asarray(norm_w, np.float32)
    s = x.shape[1]
    if s not in _NC_CACHE:
        _NC_CACHE[s] = build_nc(s)
    nc = _NC_CACHE[s]
    in_maps = make_inputs(x, position, w_qkv, w_out, norm_w, s=s)
    res = run_bass_kernel_spmd(nc, in_maps, list(range(NC)))
    return assemble(res.results, s=s)



# revision 25
# speedup vs baseline: 6.2475x; 6.2475x over previous
"""Trainium2 Bass kernel for nn_Attention_85710367359111 (v2).

Full (unsharded) numpy inputs in, full output out. Tensor-parallel over
heads (16 heads / 8 cores = 2 heads per core); each core computes QKV +
RMSNorm + RoPE + attention for its 2 heads over all 4096 tokens, then a
per-batch AllToAll re-shards to token-parallel for the output projection.

Dtypes: fp16 for all matmul operands (x, w_qkv, q̂, k̂, v, o, w_out),
bf16 for exp(scores) (range up to ~5e8 overflows fp16), fp32 PSUM accum.

Per-core phases (per batch b):
  A  qkv^T chunk = W @ x^T chunk                    (PE, fp16)
  B  RMS stats via gpsimd partition_all_reduce (pre-broadcast) ->
     ACT Rsqrt -> DVE scale -> RoPE (DVE muls + PE perm-matmul + DVE add)
  B' v -> token-major via DMA-transpose (xbar), ones cols pre-memset
  C  per 512-q chunk: scores^T per ktile (PE) -> Exp (ACT, bf16) ->
     o^T[q,65] = ex_slice^T @ [v|1] PSUM-accum over ktiles (PE) ->
     per-partition reciprocal+normalize (DVE) -> PE transpose -> osh DMA
  A2A(b)  AllToAll of osh_b [1024 x 256] fp16      (overlapped with C(b+1))
  D(b) out^T = w_out^T.T @ gathered (row-parallel)  -> PSUM -> DRAM

Emission order interleaves A/B(b1) into C(b0) and D(b0) into C(b1) so PE
stays fed while C is ACT(exp)-bound.
"""

import math
import os

import numpy as np

import concourse.bacc as bacc
import concourse.bass as bass
import concourse.tile as tile
from concourse import mybir
from concourse import bass_isa
from concourse.bass_utils import run_bass_kernel_spmd

# ---------------------------------------------------------------- config

B, S, D, H, E = 2, 2048, 1024, 16, 64
NC = 8                      # cores
HPC = H // NC               # heads per core = 2
DL = HPC * E                # local feature slice = 128
T = B * S                   # 4096 tokens
TBB = S // NC               # tokens per core per batch = 256
CH = 512                    # token chunk for phases A/B
NCHB = S // CH              # chunks per batch = 4
KT = S // 128               # k tiles per batch = 16
QC = 512                    # q chunk in phase C
NQC = S // QC               # 4
NQT = QC // 128             # q tiles per q chunk = 4
EPS = float(np.finfo(np.float32).eps)

f16 = mybir.dt.float16
bf16 = mybir.dt.bfloat16
f32 = mybir.dt.float32

EXDT = bf16                 # exp output dtype


def build_nc(s=S, reps=1):
    assert s == S, "kernel is specialized for S=2048"
    nc = bacc.Bacc("TRN2", target_bir_lowering=False, debug=False, num_devices=NC)

    # ------------- DRAM I/O
    xT_d = nc.dram_tensor("xT", [D, T], f16, kind="ExternalInput")
    wq_d = nc.dram_tensor("wqT", [D, DL], f16, kind="ExternalInput")
    wk_d = nc.dram_tensor("wkT", [D, DL], f16, kind="ExternalInput")
    wv_d = nc.dram_tensor("wvT", [D, DL], f16, kind="ExternalInput")
    wo_d = nc.dram_tensor("woT", [D, D], f16, kind="ExternalInput")
    cosw_d = nc.dram_tensor("cosw", [128, T], f16, kind="ExternalInput")
    sinp_d = nc.dram_tensor("sinp", [128, T], f16, kind="ExternalInput")
    perm_d = nc.dram_tensor("permT", [128, 128], f16, kind="ExternalInput")
    iden_d = nc.dram_tensor("iden", [128, 128], f16, kind="ExternalInput")
    sel_d = nc.dram_tensor("sel", [128, 2], f16, kind="ExternalInput")
    wsel_d = nc.dram_tensor("wsel", [2, 128], mybir.dt.float32r, kind="ExternalInput")

    DBG = bool(os.environ.get("KDEBUG"))
    dbg = {}
    if DBG:
        for nm, shp, dt_ in [("dbg_qhat", [128, S], f16), ("dbg_khat", [128, S], f16),
                             ("dbg_q01", [128, S], f16),
                             ("dbg_vtok", [128, B * KT * 130], f16),
                             ("dbg_osh0", [NC * DL, TBB], f16),
                             ("dbg_oga0", [NC * DL, TBB], f16),
                             ("dbg_ex", [128, 2 * QC], EXDT),
                             ("dbg_oh0", [128, NQT * 128], f32)]:
            dbg[nm] = nc.dram_tensor(nm, shp, dt_, kind="ExternalOutput")

    osh_d = [nc.dram_tensor(f"o_shard{b}", [NC * DL, TBB], f16) for b in range(B)]
    oga_d = [nc.dram_tensor(f"o_gath{b}", [NC * DL, TBB], f16) for b in range(B)]
    out_d = [nc.dram_tensor(f"out{b}", [D, TBB], f32, kind="ExternalOutput")
             for b in range(B)]

    xT_v = xT_d.ap().rearrange("(dt p) t -> dt p t", p=128)     # [8,128,T]
    wo_v = wo_d.ap().rearrange("(dt p) o -> dt p o", p=128)     # [8,128,D]
    oga_v = [oga_d[b].ap().rearrange("(dt p) t -> dt p t", p=128) for b in range(B)]

    from contextlib import ExitStack

    with tile.TileContext(nc) as tc, ExitStack() as ctx:
      for _rep in range(reps):
        with ExitStack() as ctx2:
            pers = ctx2.enter_context(tc.tile_pool(name="pers", bufs=1))
            wq_s = pers.tile([128, 8, 128], f16, tag="wq")
            wk_s = pers.tile([128, 8, 128], f16, tag="wk")
            wv_s = pers.tile([128, 8, 128], f16, tag="wv")
            cosw_s = pers.tile([128, T], f16, tag="cosw")
            sinp_s = pers.tile([128, T], f16, tag="sinp")
            perm_s = pers.tile([128, 128], f16, tag="perm")
            iden_s = pers.tile([128, 128], f16, tag="iden")
            sel_s = pers.tile([128, 2], f16, tag="sel")
            wsel_s = pers.tile([2, 128], mybir.dt.float32r, tag="wsel")
            epsb_s = pers.tile([2, 1], f32, tag="epsb")
            q01 = pers.tile([128, T], f16, tag="q01")   # raw q (pre-norm), f16
            k01 = pers.tile([128, T], f16, tag="k01")
            qhat = pers.tile([128, T], f16, tag="qhat")  # normed+roped
            khat = pers.tile([128, T], f16, tag="khat")
            vtok = pers.tile([128, B, KT, 130], f16, tag="vtok")
            wo_s = pers.tile([128, 8, D], f16, tag="wo")
            g_s = pers.tile([128, B, 8, TBB], f16, tag="g")

            for dt_i in range(8):
                nc.sync.dma_start(wq_s[:, dt_i, :], wq_d.ap()[bass.ts(dt_i, 128), :])
                nc.sync.dma_start(wk_s[:, dt_i, :], wk_d.ap()[bass.ts(dt_i, 128), :])
                nc.sync.dma_start(wv_s[:, dt_i, :], wv_d.ap()[bass.ts(dt_i, 128), :])
            nc.sync.dma_start(cosw_s[:], cosw_d.ap())
            nc.sync.dma_start(sinp_s[:], sinp_d.ap())
            nc.sync.dma_start(perm_s[:], perm_d.ap())
            nc.sync.dma_start(iden_s[:], iden_d.ap())
            nc.sync.dma_start(sel_s[:], sel_d.ap())
            nc.sync.dma_start(wsel_s[:], wsel_d.ap())
            nc.vector.memset(epsb_s[:], 8.0 * EPS)
            nc.vector.memset(vtok[:, :, :, 64], 1.0)
            nc.vector.memset(vtok[:, :, :, 129], 1.0)
            for dt_i in range(8):
                nc.sync.dma_start(wo_s[:, dt_i, :], wo_v[dt_i])

            xtp = ctx2.enter_context(tc.tile_pool(name="xt", bufs=3))
            psA = ctx2.enter_context(
                tc.tile_pool(name="psA", bufs=1, space=bass.MemorySpace.PSUM))
            scps = ctx2.enter_context(
                tc.tile_pool(name="scps", bufs=2, space=bass.MemorySpace.PSUM))
            ohps = ctx2.enter_context(
                tc.tile_pool(name="ohps", bufs=1, space=bass.MemorySpace.PSUM))
            bwork = ctx2.enter_context(tc.tile_pool(name="bwork", bufs=2))
            rwork = ctx2.enter_context(tc.tile_pool(name="rwork", bufs=2))
            expp = ctx2.enter_context(tc.tile_pool(name="expp", bufs=3))
            nrm = ctx2.enter_context(tc.tile_pool(name="nrm", bufs=2))

            def emit_A(b, c):
                """QKV projection chunk c: A matmuls + PSUM->SBUF copies +
                v token-major transpose."""
                tok = slice(b * S + c * CH, b * S + (c + 1) * CH)
                xt = xtp.tile([128, 8, CH], f16, tag="xt")
                for dt_i in range(8):
                    nc.sync.dma_start(xt[:, dt_i, :], xT_v[dt_i, :, tok])
                for w_s, dst, dtag in ((wq_s, q01, None), (wk_s, k01, None),
                                       (wv_s, None, "vsb")):
                    ps = psA.tile([128, CH], f32, tag="x0")
                    for dt_i in range(8):
                        nc.tensor.matmul(ps[:], w_s[:, dt_i, :], xt[:, dt_i, :],
                                         start=(dt_i == 0), stop=(dt_i == 7))
                    if dst is not None:
                        nc.vector.tensor_copy(dst[:, tok], ps[:])
                    else:
                        v_sb = bwork.tile([128, CH], f16, tag="vsb")
                        nc.vector.tensor_copy(v_sb[:], ps[:])
                # v -> token-major via DMA transpose (dense out) + strided copy
                vt0 = bwork.tile([128, NQT, 64], f16, tag="vt0")
                vt1 = bwork.tile([128, NQT, 64], f16, tag="vt1")
                nc.sync.dma_start_transpose(vt0[:], v_sb[0:64, :])
                nc.sync.dma_start_transpose(vt1[:], v_sb[64:128, :])
                kts = slice(c * NQT, (c + 1) * NQT)
                nc.vector.tensor_copy(vtok[:, b, kts, 0:64], vt0[:])
                nc.vector.tensor_copy(vtok[:, b, kts, 65:129], vt1[:])

            def emit_B(b, c):
                """RMS stats + scale + rope for chunk c of batch b."""
                tok = slice(b * S + c * CH, b * S + (c + 1) * CH)
                for src, dest in ((q01, qhat), (k01, khat)):
                    sq = bwork.tile([128, CH], f16, tag="sq")
                    nc.vector.tensor_mul(sq[:], src[:, tok], src[:, tok])
                    ms = psA.tile([2, CH], f32, tag="x1")
                    nc.tensor.matmul(ms[:], sel_s[:], sq[:], start=True, stop=True)
                    # sqv = sqrt(8*(mean+eps)) broadcast to all 128 rows first,
                    # then reciprocal -> alpha_bc; avoids f32r rounding issues
                    sqv = bwork.tile([2, CH], mybir.dt.float32r, tag="sqv")
                    nc.scalar.activation(sqv[:], ms[:],
                                         mybir.ActivationFunctionType.Sqrt,
                                         bias=epsb_s[:], scale=8.0)
                    sbc = psA.tile([128, CH], f32, tag="x1")
                    nc.tensor.matmul(sbc[:], wsel_s[:], sqv[:], start=True, stop=True)
                    abc = bwork.tile([128, CH], f32, tag="abc")
                    nc.vector.reciprocal_approx_fast(abc[:], sbc[:])
                    qs = rwork.tile([128, CH], f16, tag="qs")
                    nc.vector.tensor_mul(qs[:], src[:, tok], abc[:])
                    m1 = rwork.tile([128, CH], f16, tag="m1")
                    nc.vector.tensor_mul(m1[:], qs[:], cosw_s[:, tok])
                    m2 = rwork.tile([128, CH], f16, tag="m2")
                    nc.vector.tensor_mul(m2[:], qs[:], sinp_s[:, tok])
                    yp = psA.tile([128, CH], f32, tag="x1")
                    nc.tensor.matmul(yp[:], perm_s[:], m2[:], start=True, stop=True)
                    nc.vector.tensor_add(dest[:, tok], m1[:], yp[:])

            def phase_AB(b):
                """Software-pipelined emission: chunk c's A ahead of chunk
                c-1's B so the x1 stats chain never blocks A matmuls."""
                for c in range(NCHB):
                    emit_A(b, c)
                    if c >= 1:
                        emit_B(b, c - 1)
                emit_B(b, NCHB - 1)

            def phase_C(b, qc):
                """Attention for batch b, q chunk qc (QC tokens, both heads)."""
                qs_ = slice(b * S + qc * QC, b * S + (qc + 1) * QC)
                oh0 = ohps.tile([128, NQT, 128], f32, tag="oh0")
                oh1 = ohps.tile([128, NQT, 128], f32, tag="oh1")
                oh = [oh0, oh1]
                for kt in range(KT):
                    ks = slice(b * S + kt * 128, b * S + (kt + 1) * 128)
                    sc = scps.tile([128, 2, QC], f32, tag="sc")
                    nc.tensor.matmul(sc[:, 0, :], khat[0:64, ks], qhat[0:64, qs_],
                                     start=True, stop=True)
                    nc.tensor.matmul(sc[:, 1, :], khat[64:128, ks], qhat[64:128, qs_],
                                     start=True, stop=True)
                    ex = expp.tile([128, 2, QC], EXDT, tag="ex")
                    nc.scalar.activation(ex[:], sc[:],
                                         mybir.ActivationFunctionType.Exp)
                    if DBG and b == 0 and qc == 0 and kt == 0:
                        nc.sync.dma_start(dbg["dbg_ex"].ap(),
                                          ex[:].rearrange("p a b -> p (a b)"))
                    st_once = os.environ.get("KPSTART", "once") == "once"
                    for h in range(2):
                        for qt in range(NQT):
                            nc.tensor.matmul(
                                oh[h][:, qt, 0:65],
                                ex[:, h, bass.ts(qt, 128)],
                                vtok[:, b, kt, h * 65:h * 65 + 65],
                                start=(kt == 0 and (qt == 0 or not st_once)),
                                stop=(kt == KT - 1 and qt == NQT - 1),
                                skip_group_check=True)
                # normalize (per-partition denominators) + transpose + osh DMA
                if DBG and b == 0 and qc == 0:
                    oh0c = nrm.tile([128, NQT, 65], f32, tag="oh0c")
                    nc.vector.tensor_copy(oh0c[:], oh[0][:, :, 0:65])
                    nc.sync.dma_start(dbg["dbg_oh0"].ap()[:, 0:NQT * 65],
                                      oh0c[:].rearrange("p a b -> p (a b)"))
                tr = psA.tile([64, 8, 128], f16, tag="x0")
                for h in range(2):
                    rec = nrm.tile([128, NQT], f32, tag="rec")
                    nc.vector.reciprocal_approx_fast(rec[:], oh[h][:, :, 64])
                    of = nrm.tile([128, NQT, 64], f16, tag=f"of{h}")
                    nc.vector.tensor_mul(of[:], oh[h][:, :, 0:64],
                                         rec[:].broadcast_to([128, NQT, 64]))
                    for qt in range(NQT):
                        nc.tensor.transpose(tr[:, h * NQT + qt, :], of[:, qt, :],
                                            iden_s[:])
                trs = nrm.tile([64, 8, 128], f16, tag="trs")
                nc.vector.tensor_copy(trs[:], tr[:])
                for qt in range(NQT):
                    blk = 2 * qc + qt // 2
                    col = (qt % 2) * 128
                    for h in range(2):
                        nc.sync.dma_start(
                            osh_d[b].ap()[blk * DL + h * 64: blk * DL + h * 64 + 64,
                                          col:col + 128],
                            trs[:, h * NQT + qt, :])

            def phase_A2A(b):
                nc.gpsimd.collective_compute(
                    "AllToAll", mybir.AluOpType.bypass,
                    replica_groups=[list(range(NC))],
                    ins=[osh_d[b].ap()], outs=[oga_d[b].ap()],
                )

            def phase_D(b):
                for dt_i in range(8):
                    nc.sync.dma_start(g_s[:, b, dt_i, :], oga_v[b][dt_i])
                for do in range(8):
                    po = psA.tile([128, TBB], f32, tag="x0")
                    for dt_i in range(8):
                        nc.tensor.matmul(po[:], wo_s[:, dt_i, bass.ts(do, 128)],
                                         g_s[:, b, dt_i, :],
                                         start=(dt_i == 0), stop=(dt_i == 7))
                    ob = nrm.tile([128, TBB], f32, tag="ob")
                    nc.vector.tensor_copy(ob[:], po[:])
                    nc.sync.dma_start(out_d[b].ap()[bass.ts(do, 128), :], ob[:])

            # ---------------- emission schedule
            phase_AB(0)
            if DBG:
                nc.sync.dma_start(dbg["dbg_q01"].ap(), q01[:, 0:S])
                nc.sync.dma_start(dbg["dbg_qhat"].ap(), qhat[:, 0:S])
                nc.sync.dma_start(dbg["dbg_khat"].ap(), khat[:, 0:S])
                nc.sync.dma_start(dbg["dbg_vtok"].ap()[:, 0:KT * 130],
                                  vtok[:, 0].rearrange("p b c -> p (b c)"))
            phase_C(0, 0)
            phase_C(0, 1)
            phase_AB(1)
            phase_C(0, 2)
            phase_C(0, 3)
            phase_A2A(0)
            if DBG:
                nc.sync.dma_start(dbg["dbg_osh0"].ap(), osh_d[0].ap())
                nc.sync.dma_start(dbg["dbg_oga0"].ap(), oga_d[0].ap())
            phase_C(1, 0)
            phase_C(1, 1)
            phase_D(0)
            phase_C(1, 2)
            phase_C(1, 3)
            phase_A2A(1)
            phase_D(1)

    nc.compile()
    return nc


def make_inputs(x, position, w_qkv, w_out, norm_w, s=None):
    """Build the 8 per-core input dicts from full inputs."""
    assert (s or x.shape[1]) == S
    xT = np.ascontiguousarray(x.reshape(T, D).T).astype(np.float16)
    cos = position[0]   # [S, E]
    sin = position[1]
    nw = np.asarray(norm_w, np.float32)

    # rope permutation g: dest e<32 <- src 2e+1 (sign -1); dest 32+e <- src 2e
    g_idx = np.zeros(64, np.int64)
    sign = np.zeros(64, np.float32)
    for i in range(32):
        g_idx[i] = 2 * i + 1
        sign[i] = -1.0
        g_idx[32 + i] = 2 * i
        sign[32 + i] = 1.0

    # dest_e = qs_e*cosw_e + sign_e*m2_{g(e)} with m2_x = qs_x*sinp_x, so the
    # sin table lives in SOURCE index space: sinp_x = w_x * sin_{g^-1(x)}
    ginv = np.argsort(g_idx)
    cosw1 = (nw[None, :] * cos).T                      # [E, S]
    sinp1 = (sin[:, ginv] * nw[None, :]).T             # [E, S]
    cosw = np.tile(np.concatenate([cosw1, cosw1], 0), (1, B)).astype(np.float16)
    sinp = np.tile(np.concatenate([sinp1, sinp1], 0), (1, B)).astype(np.float16)

    # signed permutation matrix P: yp_e = sign_e * qs_{g(e)}
    P = np.zeros((64, 64), np.float32)
    for e in range(64):
        P[e, g_idx[e]] = sign[e]
    Pb = np.zeros((128, 128), np.float32)
    Pb[0:64, 0:64] = P
    Pb[64:128, 64:128] = P
    permT = np.ascontiguousarray(Pb.T).astype(np.float16)
    iden = np.eye(128, dtype=np.float16)

    woT = np.ascontiguousarray(np.asarray(w_out, np.float32).T).astype(np.float16)

    sel = np.zeros((128, 2), np.float16)
    sel[0:64, 0] = 1.0 / 64.0
    sel[64:128, 1] = 1.0 / 64.0
    wsel = np.zeros((2, 128), np.float32)
    wsel[0, 0:64] = 1.0
    wsel[1, 64:128] = 1.0

    w3 = np.asarray(w_qkv, np.float32).reshape(H, 3, E, D)
    in_maps = []
    for c in range(NC):
        h0, h1 = HPC * c, HPC * c + 1
        wqT = np.ascontiguousarray(
            np.concatenate([w3[h0, 0], w3[h1, 0]], 0).T).astype(np.float16)
        wkT = np.ascontiguousarray(
            np.concatenate([w3[h0, 1], w3[h1, 1]], 0).T).astype(np.float16)
        wvT = np.ascontiguousarray(
            np.concatenate([w3[h0, 2], w3[h1, 2]], 0).T).astype(np.float16)
        in_maps.append({
            "xT": xT, "wqT": wqT, "wkT": wkT, "wvT": wvT, "woT": woT,
            "cosw": cosw, "sinp": sinp, "permT": permT, "iden": iden,
            "sel": sel, "wsel": wsel,
        })
    return in_maps


def assemble(results, s=None):
    out = np.empty((B, S, D), np.float32)
    for c in range(NC):
        for b in range(B):
            out[b, c * TBB:(c + 1) * TBB, :] = results[c][f"out{b}"].T
    return out


_NC_CACHE = {}


def kernel(x, position, w_qkv, w_out, norm_w, heads):
    x = np.asarray(x, np.float32)
    position = np.asarray(position, np.float32)
    w_qkv = np.asarray(w_qkv, np.float32)
    w_out = np.asarray(w_out, np.float32)
    norm_w = np.asarray(norm_w, np.float32)
    s = x.shape[1]
    if s not in _NC_CACHE:
        _NC_CACHE[s] = build_nc(s)
    nc = _NC_CACHE[s]
    in_maps = make_inputs(x, position, w_qkv, w_out, norm_w, s=s)
    res = run_bass_kernel_spmd(nc, in_maps, list(range(NC)))
    return assemble(res.results, s=s)


# revision 30
# speedup vs baseline: 7.7855x; 1.2462x over previous
"""Trainium2 Bass kernel for nn_Attention_85710367359111 (v2).

Full (unsharded) numpy inputs in, full output out. Tensor-parallel over
heads (16 heads / 8 cores = 2 heads per core); each core computes QKV +
RMSNorm + RoPE + attention for its 2 heads over all 4096 tokens, then a
per-batch AllToAll re-shards to token-parallel for the output projection.

Dtypes: fp16 for all matmul operands (x, w_qkv, q̂, k̂, v, o, w_out),
bf16 for exp(scores) (range up to ~5e8 overflows fp16), fp32 PSUM accum.

Per-core phases (per batch b):
  A  qkv^T chunk = W @ x^T chunk                    (PE, fp16)
  B  RMS stats via gpsimd partition_all_reduce (pre-broadcast) ->
     ACT Rsqrt -> DVE scale -> RoPE (DVE muls + PE perm-matmul + DVE add)
  B' v -> token-major via DMA-transpose (xbar), ones cols pre-memset
  C  per 512-q chunk: scores^T per ktile (PE) -> Exp (ACT, bf16) ->
     o^T[q,65] = ex_slice^T @ [v|1] PSUM-accum over ktiles (PE) ->
     per-partition reciprocal+normalize (DVE) -> PE transpose -> osh DMA
  A2A(b)  AllToAll of osh_b [1024 x 256] fp16      (overlapped with C(b+1))
  D(b) out^T = w_out^T.T @ gathered (row-parallel)  -> PSUM -> DRAM

Emission order interleaves A/B(b1) into C(b0) and D(b0) into C(b1) so PE
stays fed while C is ACT(exp)-bound.
"""

import math
import os

import numpy as np

import concourse.bacc as bacc
import concourse.bass as bass
import concourse.tile as tile
from concourse import mybir
from concourse import bass_isa
from concourse.bass_utils import run_bass_kernel_spmd

# ---------------------------------------------------------------- config

B, S, D, H, E = 2, 2048, 1024, 16, 64
NC = 8                      # cores
HPC = H // NC               # heads per core = 2
DL = HPC * E                # local feature slice = 128
T = B * S                   # 4096 tokens
TBB = S // NC               # tokens per core per batch = 256
CH = 512                    # token chunk for phases A/B
NCHB = S // CH              # chunks per batch = 4
KT = S // 128               # k tiles per batch = 16
QC = 512                    # q chunk in phase C
NQC = S // QC               # 4
NQT = QC // 128             # q tiles per q chunk = 4
EPS = float(np.finfo(np.float32).eps)

f16 = mybir.dt.float16
bf16 = mybir.dt.bfloat16
f32 = mybir.dt.float32

EXDT = bf16                 # exp output dtype


def build_nc(s=S, reps=1):
    assert s == S, "kernel is specialized for S=2048"
    nc = bacc.Bacc("TRN2", target_bir_lowering=False, debug=False, num_devices=NC)

    # ------------- DRAM I/O
    xT_d = nc.dram_tensor("xT", [D, T], f16, kind="ExternalInput")
    wq_d = nc.dram_tensor("wqT", [D, DL], f16, kind="ExternalInput")
    wk_d = nc.dram_tensor("wkT", [D, DL], f16, kind="ExternalInput")
    wv_d = nc.dram_tensor("wvT", [D, DL], f16, kind="ExternalInput")
    wo_d = nc.dram_tensor("woT", [D, D], f16, kind="ExternalInput")
    cosw_d = nc.dram_tensor("cosw", [128, T], f16, kind="ExternalInput")
    sinp_d = nc.dram_tensor("sinp", [128, T], f16, kind="ExternalInput")
    perm_d = nc.dram_tensor("permT", [128, 128], f16, kind="ExternalInput")
    iden_d = nc.dram_tensor("iden", [128, 128], f16, kind="ExternalInput")
    sel_d = nc.dram_tensor("sel", [128, 2], f16, kind="ExternalInput")
    wsel_d = nc.dram_tensor("wsel", [2, 128], mybir.dt.float32r, kind="ExternalInput")

    DBG = bool(os.environ.get("KDEBUG"))
    dbg = {}
    if DBG:
        for nm, shp, dt_ in [("dbg_qhat", [128, S], f16), ("dbg_khat", [128, S], f16),
                             ("dbg_q01", [128, S], f16),
                             ("dbg_vtok", [128, B * KT * 130], f16),
                             ("dbg_osh0", [NC * DL, TBB], f16),
                             ("dbg_oga0", [NC * DL, TBB], f16),
                             ("dbg_ex", [128, 2 * QC], EXDT),
                             ("dbg_oh0", [128, NQT * 128], f32)]:
            dbg[nm] = nc.dram_tensor(nm, shp, dt_, kind="ExternalOutput")

    osh_d = [nc.dram_tensor(f"o_shard{b}", [NC * DL, TBB], f16) for b in range(B)]
    oga_d = [nc.dram_tensor(f"o_gath{b}", [NC * DL, TBB], f16) for b in range(B)]
    out_d = [nc.dram_tensor(f"out{b}", [D, TBB], f32, kind="ExternalOutput")
             for b in range(B)]

    xT_v = xT_d.ap().rearrange("(dt p) t -> dt p t", p=128)     # [8,128,T]
    wo_v = wo_d.ap().rearrange("(dt p) o -> dt p o", p=128)     # [8,128,D]
    oga_v = [oga_d[b].ap().rearrange("(dt p) t -> dt p t", p=128) for b in range(B)]

    from contextlib import ExitStack

    with tile.TileContext(nc) as tc, ExitStack() as ctx:
      for _rep in range(reps):
        with ExitStack() as ctx2:
            pers = ctx2.enter_context(tc.tile_pool(name="pers", bufs=1))
            wq_s = pers.tile([128, 8, 128], f16, tag="wq")
            wk_s = pers.tile([128, 8, 128], f16, tag="wk")
            wv_s = pers.tile([128, 8, 128], f16, tag="wv")
            cosw_s = pers.tile([128, T], f16, tag="cosw")
            sinp_s = pers.tile([128, T], f16, tag="sinp")
            perm_s = pers.tile([128, 128], f16, tag="perm")
            iden_s = pers.tile([128, 128], f16, tag="iden")
            sel_s = pers.tile([128, 2], f16, tag="sel")
            wsel_s = pers.tile([2, 128], mybir.dt.float32r, tag="wsel")
            epsb_s = pers.tile([2, 1], f32, tag="epsb")
            q01 = pers.tile([128, T], f16, tag="q01")   # raw q (pre-norm), f16
            k01 = pers.tile([128, T], f16, tag="k01")
            qhat = pers.tile([128, T], f16, tag="qhat")  # normed+roped
            khat = pers.tile([128, T], f16, tag="khat")
            vtok = pers.tile([128, B, KT, 130], f16, tag="vtok")
            wo_s = pers.tile([128, 8, D], f16, tag="wo")
            g_s = pers.tile([128, B, 8, TBB], f16, tag="g")

            # qkv weights first so chunk-0 A matmuls can start immediately;
            # rope tables stream in under the A compute; wo loads before D.
            for dt_i in range(8):
                nc.sync.dma_start(wq_s[:, dt_i, :], wq_d.ap()[bass.ts(dt_i, 128), :])
                nc.sync.dma_start(wk_s[:, dt_i, :], wk_d.ap()[bass.ts(dt_i, 128), :])
                nc.sync.dma_start(wv_s[:, dt_i, :], wv_d.ap()[bass.ts(dt_i, 128), :])
            nc.sync.dma_start(perm_s[:], perm_d.ap())
            nc.sync.dma_start(iden_s[:], iden_d.ap())
            nc.sync.dma_start(sel_s[:], sel_d.ap())
            nc.sync.dma_start(wsel_s[:], wsel_d.ap())
            nc.vector.memset(epsb_s[:], 8.0 * EPS)
            nc.vector.memset(vtok[:, :, :, 64], 1.0)
            nc.vector.memset(vtok[:, :, :, 129], 1.0)

            xtp = ctx2.enter_context(tc.tile_pool(name="xt", bufs=3))
            psA = ctx2.enter_context(
                tc.tile_pool(name="psA", bufs=1, space=bass.MemorySpace.PSUM))
            scps = ctx2.enter_context(
                tc.tile_pool(name="scps", bufs=2, space=bass.MemorySpace.PSUM))
            ohps = ctx2.enter_context(
                tc.tile_pool(name="ohps", bufs=1, space=bass.MemorySpace.PSUM))
            bwork = ctx2.enter_context(tc.tile_pool(name="bwork", bufs=2))
            rwork = ctx2.enter_context(tc.tile_pool(name="rwork", bufs=2))
            expp = ctx2.enter_context(tc.tile_pool(name="expp", bufs=3))
            nrm = ctx2.enter_context(tc.tile_pool(name="nrm", bufs=2))

            def emit_A(b, c):
                """QKV projection chunk c: A matmuls + PSUM->SBUF copies +
                v token-major transpose."""
                tok = slice(b * S + c * CH, b * S + (c + 1) * CH)
                xt = xtp.tile([128, 8, CH], f16, tag="xt")
                for dt_i in range(8):
                    nc.sync.dma_start(xt[:, dt_i, :], xT_v[dt_i, :, tok])
                for w_s, dst, dtag in ((wq_s, q01, None), (wk_s, k01, None),
                                       (wv_s, None, "vsb")):
                    ps = psA.tile([128, CH], f32, tag="x0")
                    for dt_i in range(8):
                        nc.tensor.matmul(ps[:], w_s[:, dt_i, :], xt[:, dt_i, :],
                                         start=(dt_i == 0), stop=(dt_i == 7))
                    if dst is not None:
                        nc.vector.tensor_copy(dst[:, tok], ps[:])
                    else:
                        v_sb = bwork.tile([128, CH], f16, tag="vsb")
                        nc.vector.tensor_copy(v_sb[:], ps[:])
                # v -> token-major via DMA transpose (dense out) + strided copy
                vt0 = bwork.tile([128, NQT, 64], f16, tag="vt0")
                vt1 = bwork.tile([128, NQT, 64], f16, tag="vt1")
                nc.sync.dma_start_transpose(vt0[:], v_sb[0:64, :])
                nc.sync.dma_start_transpose(vt1[:], v_sb[64:128, :])
                kts = slice(c * NQT, (c + 1) * NQT)
                nc.vector.tensor_copy(vtok[:, b, kts, 0:64], vt0[:])
                nc.vector.tensor_copy(vtok[:, b, kts, 65:129], vt1[:])

            def emit_rope(b, c):
                """RoPE on RAW q,k (independent of the RMS stats): writes
                qhat/khat = rope(q01/k01); the alpha scale multiplies later
                (rope and per-token scaling commute)."""
                tok = slice(b * S + c * CH, b * S + (c + 1) * CH)
                for src, dest in ((q01, qhat), (k01, khat)):
                    m1 = rwork.tile([128, CH], f16, tag="m1")
                    nc.vector.tensor_mul(m1[:], src[:, tok], cosw_s[:, tok])
                    m2 = rwork.tile([128, CH], f16, tag="m2")
                    nc.vector.tensor_mul(m2[:], src[:, tok], sinp_s[:, tok])
                    yp = psA.tile([128, CH], f32, tag="x1")
                    nc.tensor.matmul(yp[:], perm_s[:], m2[:], start=True, stop=True)
                    nc.vector.tensor_add(dest[:, tok], m1[:], yp[:])

            def emit_stats(b, c, mss):
                """Sum-of-squares stats for chunk c; chunk 0 keeps its ms in
                PSUM (solo sqrt), chunks 1..3 copy ms into mss for one
                batched sqrt (ACT Copy never switches tables)."""
                tok = slice(b * S + c * CH, b * S + (c + 1) * CH)
                out = []
                for ti, src in enumerate((q01, k01)):
                    sq = bwork.tile([128, CH], f16, tag="sq")
                    nc.vector.tensor_mul(sq[:], src[:, tok], src[:, tok])
                    ms = psA.tile([2, CH], f32, tag="x1")
                    nc.tensor.matmul(ms[:], sel_s[:], sq[:], start=True, stop=True)
                    if c == 0:
                        sqv = bwork.tile([2, CH], mybir.dt.float32r, tag="sqv")
                        nc.scalar.activation(sqv[:], ms[:],
                                             mybir.ActivationFunctionType.Sqrt,
                                             bias=epsb_s[:], scale=8.0)
                        out.append(sqv[:])
                    else:
                        j = 2 * (c - 1) + ti
                        nc.scalar.copy(mss[:, j, :], ms[:])
                        out.append(None)
                return out

            def emit_scale(b, c, sqv_q, sqv_k):
                """alpha broadcast + final in-place scale of qhat/khat."""
                tok = slice(b * S + c * CH, b * S + (c + 1) * CH)
                for sqv, dest in ((sqv_q, qhat), (sqv_k, khat)):
                    sbc = psA.tile([128, CH], f32, tag="x1")
                    nc.tensor.matmul(sbc[:], wsel_s[:], sqv, start=True, stop=True)
                    abc = bwork.tile([128, CH], f32, tag="abc")
                    nc.vector.reciprocal_approx_fast(abc[:], sbc[:])
                    nc.vector.tensor_mul(dest[:, tok], dest[:, tok], abc[:])

            def phase_AB(b):
                """Chunk 0 finalizes immediately (so phase C's first ktiles
                can start); chunks 1..3 share one batched sqrt."""
                mss = bwork.tile([2, 2 * (NCHB - 1), CH], f32, tag="mss")
                sqv123 = bwork.tile([2, 2 * (NCHB - 1), CH],
                                    mybir.dt.float32r, tag="sqv123")
                for c in range(NCHB):
                    emit_A(b, c)
                    if b == 0 and c == 0:
                        # rope tables stream under chunk-0 A compute
                        tok0 = slice(0, S)
                        nc.sync.dma_start(cosw_s[:, tok0], cosw_d.ap()[:, tok0])
                        nc.sync.dma_start(sinp_s[:, tok0], sinp_d.ap()[:, tok0])
                    if b == 0 and c == 1:
                        tok1 = slice(S, T)
                        nc.sync.dma_start(cosw_s[:, tok1], cosw_d.ap()[:, tok1])
                        nc.sync.dma_start(sinp_s[:, tok1], sinp_d.ap()[:, tok1])
                    emit_rope(b, c)
                    sq_out = emit_stats(b, c, mss)
                    if c == 0:
                        emit_scale(b, 0, sq_out[0], sq_out[1])
                nc.scalar.activation(sqv123[:], mss[:],
                                     mybir.ActivationFunctionType.Sqrt,
                                     bias=epsb_s[:], scale=8.0)
                for c in range(1, NCHB):
                    emit_scale(b, c, sqv123[:, 2 * (c - 1), :],
                               sqv123[:, 2 * (c - 1) + 1, :])

            def phase_C(b, qc):
                """Attention for batch b, q chunk qc (QC tokens, both heads)."""
                qs_ = slice(b * S + qc * QC, b * S + (qc + 1) * QC)
                oh0 = ohps.tile([128, NQT, 128], f32, tag="oh0")
                oh1 = ohps.tile([128, NQT, 128], f32, tag="oh1")
                oh = [oh0, oh1]
                for kt in range(KT):
                    ks = slice(b * S + kt * 128, b * S + (kt + 1) * 128)
                    sc = scps.tile([128, 2, QC], f32, tag="sc")
                    nc.tensor.matmul(sc[:, 0, :], khat[0:64, ks], qhat[0:64, qs_],
                                     start=True, stop=True)
                    nc.tensor.matmul(sc[:, 1, :], khat[64:128, ks], qhat[64:128, qs_],
                                     start=True, stop=True)
                    ex = expp.tile([128, 2, QC], EXDT, tag="ex")
                    nc.scalar.activation(ex[:], sc[:],
                                         mybir.ActivationFunctionType.Exp)
                    if DBG and b == 0 and qc == 0 and kt == 0:
                        nc.sync.dma_start(dbg["dbg_ex"].ap(),
                                          ex[:].rearrange("p a b -> p (a b)"))
                    st_once = os.environ.get("KPSTART", "once") == "once"
                    for h in range(2):
                        for qt in range(NQT):
                            nc.tensor.matmul(
                                oh[h][:, qt, 0:65],
                                ex[:, h, bass.ts(qt, 128)],
                                vtok[:, b, kt, h * 65:h * 65 + 65],
                                start=(kt == 0 and (qt == 0 or not st_once)),
                                stop=(kt == KT - 1 and qt == NQT - 1),
                                skip_group_check=True)
                # normalize (per-partition denominators) + transpose + osh DMA
                if DBG and b == 0 and qc == 0:
                    oh0c = nrm.tile([128, NQT, 65], f32, tag="oh0c")
                    nc.vector.tensor_copy(oh0c[:], oh[0][:, :, 0:65])
                    nc.sync.dma_start(dbg["dbg_oh0"].ap()[:, 0:NQT * 65],
                                      oh0c[:].rearrange("p a b -> p (a b)"))
                tr = psA.tile([64, 8, 128], f16, tag="x0")
                for h in range(2):
                    rec = nrm.tile([128, NQT], f32, tag="rec")
                    nc.vector.reciprocal_approx_fast(rec[:], oh[h][:, :, 64])
                    of = nrm.tile([128, NQT, 64], f16, tag=f"of{h}")
                    nc.vector.tensor_mul(of[:], oh[h][:, :, 0:64],
                                         rec[:].broadcast_to([128, NQT, 64]))
                    for qt in range(NQT):
                        nc.tensor.transpose(tr[:, h * NQT + qt, :], of[:, qt, :],
                                            iden_s[:])
                trs = nrm.tile([64, 8, 128], f16, tag="trs")
                nc.vector.tensor_copy(trs[:], tr[:])
                for qt in range(NQT):
                    blk = 2 * qc + qt // 2
                    col = (qt % 2) * 128
                    for h in range(2):
                        nc.sync.dma_start(
                            osh_d[b].ap()[blk * DL + h * 64: blk * DL + h * 64 + 64,
                                          col:col + 128],
                            trs[:, h * NQT + qt, :])

            def phase_A2A(b):
                nc.gpsimd.collective_compute(
                    "AllToAll", mybir.AluOpType.bypass,
                    replica_groups=[list(range(NC))],
                    ins=[osh_d[b].ap()], outs=[oga_d[b].ap()],
                )

            def phase_D(b):
                if b == 0:
                    for dt_i in range(8):
                        nc.sync.dma_start(wo_s[:, dt_i, :], wo_v[dt_i])
                for dt_i in range(8):
                    nc.sync.dma_start(g_s[:, b, dt_i, :], oga_v[b][dt_i])
                for do in range(8):
                    po = psA.tile([128, TBB], f32, tag="x0")
                    for dt_i in range(8):
                        nc.tensor.matmul(po[:], wo_s[:, dt_i, bass.ts(do, 128)],
                                         g_s[:, b, dt_i, :],
                                         start=(dt_i == 0), stop=(dt_i == 7))
                    ob = nrm.tile([128, TBB], f32, tag="ob")
                    nc.vector.tensor_copy(ob[:], po[:])
                    nc.sync.dma_start(out_d[b].ap()[bass.ts(do, 128), :], ob[:])

            # ---------------- emission schedule
            phase_AB(0)
            if DBG:
                nc.sync.dma_start(dbg["dbg_q01"].ap(), q01[:, 0:S])
                nc.sync.dma_start(dbg["dbg_qhat"].ap(), qhat[:, 0:S])
                nc.sync.dma_start(dbg["dbg_khat"].ap(), khat[:, 0:S])
                nc.sync.dma_start(dbg["dbg_vtok"].ap()[:, 0:KT * 130],
                                  vtok[:, 0].rearrange("p b c -> p (b c)"))
            phase_C(0, 0)
            phase_C(0, 1)
            phase_AB(1)
            phase_C(0, 2)
            phase_C(0, 3)
            phase_A2A(0)
            if DBG:
                nc.sync.dma_start(dbg["dbg_osh0"].ap(), osh_d[0].ap())
                nc.sync.dma_start(dbg["dbg_oga0"].ap(), oga_d[0].ap())
            phase_C(1, 0)
            phase_C(1, 1)
            phase_D(0)
            phase_C(1, 2)
            phase_C(1, 3)
            phase_A2A(1)
            phase_D(1)

    nc.compile()
    return nc


def make_inputs(x, position, w_qkv, w_out, norm_w, s=None):
    """Build the 8 per-core input dicts from full inputs."""
    assert (s or x.shape[1]) == S
    xT = np.ascontiguousarray(x.reshape(T, D).T).astype(np.float16)
    cos = position[0]   # [S, E]
    sin = position[1]
    nw = np.asarray(norm_w, np.float32)

    # rope permutation g: dest e<32 <- src 2e+1 (sign -1); dest 32+e <- src 2e
    g_idx = np.zeros(64, np.int64)
    sign = np.zeros(64, np.float32)
    for i in range(32):
        g_idx[i] = 2 * i + 1
        sign[i] = -1.0
        g_idx[32 + i] = 2 * i
        sign[32 + i] = 1.0

    # dest_e = qs_e*cosw_e + sign_e*m2_{g(e)} with m2_x = qs_x*sinp_x, so the
    # sin table lives in SOURCE index space: sinp_x = w_x * sin_{g^-1(x)}
    ginv = np.argsort(g_idx)
    cosw1 = (nw[None, :] * cos).T                      # [E, S]
    sinp1 = (sin[:, ginv] * nw[None, :]).T             # [E, S]
    cosw = np.tile(np.concatenate([cosw1, cosw1], 0), (1, B)).astype(np.float16)
    sinp = np.tile(np.concatenate([sinp1, sinp1], 0), (1, B)).astype(np.float16)

    # signed permutation matrix P: yp_e = sign_e * qs_{g(e)}
    P = np.zeros((64, 64), np.float32)
    for e in range(64):
        P[e, g_idx[e]] = sign[e]
    Pb = np.zeros((128, 128), np.float32)
    Pb[0:64, 0:64] = P
    Pb[64:128, 64:128] = P
    permT = np.ascontiguousarray(Pb.T).astype(np.float16)
    iden = np.eye(128, dtype=np.float16)

    woT = np.ascontiguousarray(np.asarray(w_out, np.float32).T).astype(np.float16)

    sel = np.zeros((128, 2), np.float16)
    sel[0:64, 0] = 1.0 / 64.0
    sel[64:128, 1] = 1.0 / 64.0
    wsel = np.zeros((2, 128), np.float32)
    wsel[0, 0:64] = 1.0
    wsel[1, 64:128] = 1.0

    w3 = np.asarray(w_qkv, np.float32).reshape(H, 3, E, D)
    in_maps = []
    for c in range(NC):
        h0, h1 = HPC * c, HPC * c + 1
        wqT = np.ascontiguousarray(
            np.concatenate([w3[h0, 0], w3[h1, 0]], 0).T).astype(np.float16)
        wkT = np.ascontiguousarray(
            np.concatenate([w3[h0, 1], w3[h1, 1]], 0).T).astype(np.float16)
        wvT = np.ascontiguousarray(
            np.concatenate([w3[h0, 2], w3[h1, 2]], 0).T).astype(np.float16)
        in_maps.append({
            "xT": xT, "wqT": wqT, "wkT": wkT, "wvT": wvT, "woT": woT,
            "cosw": cosw, "sinp": sinp, "permT": permT, "iden": iden,
            "sel": sel, "wsel": wsel,
        })
    return in_maps


def assemble(results, s=None):
    out = np.empty((B, S, D), np.float32)
    for c in range(NC):
        for b in range(B):
            out[b, c * TBB:(c + 1) * TBB, :] = results[c][f"out{b}"].T
    return out


_NC_CACHE = {}


def kernel(x, position, w_qkv, w_out, norm_w, heads):
    x = np.asarray(x, np.float32)
    position = np.asarray(position, np.float32)
    w_qkv = np.asarray(w_qkv, np.float32)
    w_out = np.asarray(w_out, np.float32)
    norm_w = np.asarray(norm_w, np.float32)
    s = x.shape[1]
    if s not in _NC_CACHE:
        _NC_CACHE[s] = build_nc(s)
    nc = _NC_CACHE[s]
    in_maps = make_inputs(x, position, w_qkv, w_out, norm_w, s=s)
    res = run_bass_kernel_spmd(nc, in_maps, list(range(NC)))
    return assemble(res.results, s=s)


# revision 34
# speedup vs baseline: 9.0942x; 1.1681x over previous
"""Trainium2 Bass kernel for nn_Attention_85710367359111 (v2).

Full (unsharded) numpy inputs in, full output out. Tensor-parallel over
heads (16 heads / 8 cores = 2 heads per core); each core computes QKV +
RMSNorm + RoPE + attention for its 2 heads over all 4096 tokens, then a
per-batch AllToAll re-shards to token-parallel for the output projection.

Dtypes: fp16 for all matmul operands (x, w_qkv, q̂, k̂, v, o, w_out),
bf16 for exp(scores) (range up to ~5e8 overflows fp16), fp32 PSUM accum.

Per-core phases (per batch b):
  A  qkv^T chunk = W @ x^T chunk                    (PE, fp16)
  B  RMS stats via gpsimd partition_all_reduce (pre-broadcast) ->
     ACT Rsqrt -> DVE scale -> RoPE (DVE muls + PE perm-matmul + DVE add)
  B' v -> token-major via DMA-transpose (xbar), ones cols pre-memset
  C  per 512-q chunk: scores^T per ktile (PE) -> Exp (ACT, bf16) ->
     o^T[q,65] = ex_slice^T @ [v|1] PSUM-accum over ktiles (PE) ->
     per-partition reciprocal+normalize (DVE) -> PE transpose -> osh DMA
  A2A(b)  AllToAll of osh_b [1024 x 256] fp16      (overlapped with C(b+1))
  D(b) out^T = w_out^T.T @ gathered (row-parallel)  -> PSUM -> DRAM

Emission order interleaves A/B(b1) into C(b0) and D(b0) into C(b1) so PE
stays fed while C is ACT(exp)-bound.
"""

import math
import os

import numpy as np

import concourse.bacc as bacc
import concourse.bass as bass
import concourse.tile as tile
from concourse import mybir
from concourse import bass_isa
from concourse.bass_utils import run_bass_kernel_spmd

# ---------------------------------------------------------------- config

B, S, D, H, E = 2, 2048, 1024, 16, 64
NC = 8                      # cores
HPC = H // NC               # heads per core = 2
DL = HPC * E                # local feature slice = 128
T = B * S                   # 4096 tokens
TBB = S // NC               # tokens per core per batch = 256
CH = 512                    # token chunk for phases A/B
NCHB = S // CH              # chunks per batch = 4
KT = S // 128               # k tiles per batch = 16
QC = 512                    # q chunk in phase C
NQC = S // QC               # 4
NQT = QC // 128             # q tiles per q chunk = 4
EPS = float(np.finfo(np.float32).eps)

f16 = mybir.dt.float16
bf16 = mybir.dt.bfloat16
f32 = mybir.dt.float32

EXDT = bf16                 # exp output dtype


def build_nc(s=S, reps=1):
    assert s == S, "kernel is specialized for S=2048"
    nc = bacc.Bacc("TRN2", target_bir_lowering=False, debug=False, num_devices=NC)

    # ------------- DRAM I/O
    xT_d = nc.dram_tensor("xT", [D, T], f16, kind="ExternalInput")
    wq_d = nc.dram_tensor("wqT", [D, DL], f16, kind="ExternalInput")
    wk_d = nc.dram_tensor("wkT", [D, DL], f16, kind="ExternalInput")
    wv_d = nc.dram_tensor("wvT", [D, DL], f16, kind="ExternalInput")
    wo_d = nc.dram_tensor("woT", [D, D], f16, kind="ExternalInput")
    cosw_d = nc.dram_tensor("cosw", [128, T], f16, kind="ExternalInput")
    sinp_d = nc.dram_tensor("sinp", [128, T], f16, kind="ExternalInput")
    perm_d = nc.dram_tensor("permT", [128, 128], f16, kind="ExternalInput")
    iden_d = nc.dram_tensor("iden", [128, 128], f16, kind="ExternalInput")
    sel_d = nc.dram_tensor("sel", [128, 2], f16, kind="ExternalInput")
    wsel_d = nc.dram_tensor("wsel", [2, 128], mybir.dt.float32r, kind="ExternalInput")

    DBG = bool(os.environ.get("KDEBUG"))
    dbg = {}
    if DBG:
        for nm, shp, dt_ in [("dbg_qhat", [128, S], f16), ("dbg_khat", [128, S], f16),
                             ("dbg_q01", [128, S], f16),
                             ("dbg_vtok", [128, B * KT * 130], f16),
                             ("dbg_osh0", [NC * DL, TBB], f16),
                             ("dbg_oga0", [NC * DL, TBB], f16),
                             ("dbg_ex", [128, 2 * QC], EXDT),
                             ("dbg_oh0", [128, NQT * 128], f32)]:
            dbg[nm] = nc.dram_tensor(nm, shp, dt_, kind="ExternalOutput")

    osh_d = [nc.dram_tensor(f"o_shard{b}", [NC * DL, TBB], f16) for b in range(B)]
    oga_d = [nc.dram_tensor(f"o_gath{b}", [NC * DL, TBB], f16) for b in range(B)]
    out_d = [nc.dram_tensor(f"out{b}", [D, TBB], f32, kind="ExternalOutput")
             for b in range(B)]

    xT_p = xT_d.ap().rearrange("(a p) t -> p a t", p=128)       # [128,8,T]
    wo_p = wo_d.ap().rearrange("(a p) o -> p a o", p=128)       # [128,8,D]
    oga_p = [oga_d[b].ap().rearrange("(a p) t -> p a t", p=128) for b in range(B)]

    from contextlib import ExitStack

    with tile.TileContext(nc) as tc, ExitStack() as ctx:
      for _rep in range(reps):
        with ExitStack() as ctx2:
            pers = ctx2.enter_context(tc.tile_pool(name="pers", bufs=1))
            wq_s = pers.tile([128, 8, 128], f16, tag="wq")
            wk_s = pers.tile([128, 8, 128], f16, tag="wk")
            wv_s = pers.tile([128, 8, 128], f16, tag="wv")
            cosw_s = pers.tile([128, T], f16, tag="cosw")
            sinp_s = pers.tile([128, T], f16, tag="sinp")
            perm_s = pers.tile([128, 128], f16, tag="perm")
            iden_s = pers.tile([128, 128], f16, tag="iden")
            sel_s = pers.tile([128, 2], f16, tag="sel")
            wsel_s = pers.tile([2, 128], mybir.dt.float32r, tag="wsel")
            epsb_s = pers.tile([2, 1], f32, tag="epsb")
            q01 = pers.tile([128, T], f16, tag="q01")   # raw q (pre-norm), f16
            k01 = pers.tile([128, T], f16, tag="k01")
            qhat = pers.tile([128, T], f16, tag="qhat")  # normed+roped
            khat = pers.tile([128, T], f16, tag="khat")
            vtok = pers.tile([128, B, KT, 130], f16, tag="vtok")
            wo_s = pers.tile([128, 8, D], f16, tag="wo")
            g_s = pers.tile([128, B, 8, TBB], f16, tag="g")

            # qkv weights first so chunk-0 A matmuls can start immediately;
            # rope tables stream in under the A compute; wo loads before D.
            nc.sync.dma_start(wq_s[:], wq_d.ap().rearrange("(a p) c -> p a c", p=128))
            nc.sync.dma_start(wk_s[:], wk_d.ap().rearrange("(a p) c -> p a c", p=128))
            nc.sync.dma_start(wv_s[:], wv_d.ap().rearrange("(a p) c -> p a c", p=128))
            nc.sync.dma_start(perm_s[:], perm_d.ap())
            nc.sync.dma_start(iden_s[:], iden_d.ap())
            nc.sync.dma_start(sel_s[:], sel_d.ap())
            nc.sync.dma_start(wsel_s[:], wsel_d.ap())
            nc.vector.memset(epsb_s[:], 8.0 * EPS)
            nc.vector.memset(vtok[:, :, :, 64], 1.0)
            nc.vector.memset(vtok[:, :, :, 129], 1.0)

            xtp = ctx2.enter_context(tc.tile_pool(name="xt", bufs=3))
            psA = ctx2.enter_context(
                tc.tile_pool(name="psA", bufs=1, space=bass.MemorySpace.PSUM))
            scps = ctx2.enter_context(
                tc.tile_pool(name="scps", bufs=2, space=bass.MemorySpace.PSUM))
            ohps = ctx2.enter_context(
                tc.tile_pool(name="ohps", bufs=1, space=bass.MemorySpace.PSUM))
            bwork = ctx2.enter_context(tc.tile_pool(name="bwork", bufs=2))
            rwork = ctx2.enter_context(tc.tile_pool(name="rwork", bufs=2))
            expp = ctx2.enter_context(tc.tile_pool(name="expp", bufs=3))
            nrm = ctx2.enter_context(tc.tile_pool(name="nrm", bufs=2))

            def emit_A(b, c):
                """QKV projection chunk c: A matmuls + PSUM->SBUF copies +
                v token-major transpose."""
                tok = slice(b * S + c * CH, b * S + (c + 1) * CH)
                xt = xtp.tile([128, 8, CH], f16, tag="xt")
                nc.sync.dma_start(xt[:], xT_p[:, :, tok])
                for w_s, dst, dtag in ((wq_s, q01, None), (wk_s, k01, None),
                                       (wv_s, None, "vsb")):
                    ps = psA.tile([128, CH], f32, tag="x0")
                    for dt_i in range(8):
                        nc.tensor.matmul(ps[:], w_s[:, dt_i, :], xt[:, dt_i, :],
                                         start=(dt_i == 0), stop=(dt_i == 7))
                    if dst is not None:
                        nc.vector.tensor_copy(dst[:, tok], ps[:])
                    else:
                        v_sb = bwork.tile([128, CH], f16, tag="vsb")
                        nc.vector.tensor_copy(v_sb[:], ps[:])
                # v -> token-major via DMA transpose (dense out) + strided copy
                vt0 = bwork.tile([128, NQT, 64], f16, tag="vt0")
                vt1 = bwork.tile([128, NQT, 64], f16, tag="vt1")
                nc.sync.dma_start_transpose(vt0[:], v_sb[0:64, :])
                nc.sync.dma_start_transpose(vt1[:], v_sb[64:128, :])
                kts = slice(c * NQT, (c + 1) * NQT)
                nc.vector.tensor_copy(vtok[:, b, kts, 0:64], vt0[:])
                nc.vector.tensor_copy(vtok[:, b, kts, 65:129], vt1[:])

            def emit_rope(b, c):
                """RoPE on RAW q,k (independent of the RMS stats): writes
                qhat/khat = rope(q01/k01); the alpha scale multiplies later
                (rope and per-token scaling commute)."""
                tok = slice(b * S + c * CH, b * S + (c + 1) * CH)
                for src, dest in ((q01, qhat), (k01, khat)):
                    m1 = rwork.tile([128, CH], f16, tag="m1")
                    nc.vector.tensor_mul(m1[:], src[:, tok], cosw_s[:, tok])
                    m2 = rwork.tile([128, CH], f16, tag="m2")
                    nc.vector.tensor_mul(m2[:], src[:, tok], sinp_s[:, tok])
                    yp = psA.tile([128, CH], f32, tag="x1")
                    nc.tensor.matmul(yp[:], perm_s[:], m2[:], start=True, stop=True)
                    nc.vector.tensor_add(dest[:, tok], m1[:], yp[:])

            def emit_stats(b, c, mss):
                """Sum-of-squares stats for chunk c; chunk 0 keeps its ms in
                PSUM (solo sqrt), chunks 1..3 copy ms into mss for one
                batched sqrt (ACT Copy never switches tables)."""
                tok = slice(b * S + c * CH, b * S + (c + 1) * CH)
                out = []
                for ti, src in enumerate((q01, k01)):
                    sq = bwork.tile([128, CH], f16, tag="sq")
                    nc.vector.tensor_mul(sq[:], src[:, tok], src[:, tok])
                    ms = psA.tile([2, CH], f32, tag="x1")
                    nc.tensor.matmul(ms[:], sel_s[:], sq[:], start=True, stop=True)
                    if c == 0:
                        sqv = bwork.tile([2, CH], mybir.dt.float32r, tag="sqv")
                        nc.scalar.activation(sqv[:], ms[:],
                                             mybir.ActivationFunctionType.Sqrt,
                                             bias=epsb_s[:], scale=8.0)
                        out.append(sqv[:])
                    else:
                        j = 2 * (c - 1) + ti
                        nc.scalar.copy(mss[:, j, :], ms[:])
                        out.append(None)
                return out

            def emit_scale(b, c, sqv_q, sqv_k):
                """alpha broadcast + final in-place scale of qhat/khat."""
                tok = slice(b * S + c * CH, b * S + (c + 1) * CH)
                for sqv, dest in ((sqv_q, qhat), (sqv_k, khat)):
                    sbc = psA.tile([128, CH], f32, tag="x1")
                    nc.tensor.matmul(sbc[:], wsel_s[:], sqv, start=True, stop=True)
                    abc = bwork.tile([128, CH], f32, tag="abc")
                    nc.vector.reciprocal_approx_fast(abc[:], sbc[:])
                    nc.vector.tensor_mul(dest[:, tok], dest[:, tok], abc[:])

            def phase_AB(b):
                """Chunk 0 finalizes immediately (so phase C's first ktiles
                can start); chunks 1..3 share one batched sqrt."""
                mss = bwork.tile([2, 2 * (NCHB - 1), CH], f32, tag="mss")
                sqv123 = bwork.tile([2, 2 * (NCHB - 1), CH],
                                    mybir.dt.float32r, tag="sqv123")
                for c in range(NCHB):
                    emit_A(b, c)
                    if b == 0 and c == 0:
                        # rope tables stream under chunk-0 A compute
                        tok0 = slice(0, S)
                        nc.sync.dma_start(cosw_s[:, tok0], cosw_d.ap()[:, tok0])
                        nc.sync.dma_start(sinp_s[:, tok0], sinp_d.ap()[:, tok0])
                    if b == 0 and c == 1:
                        tok1 = slice(S, T)
                        nc.sync.dma_start(cosw_s[:, tok1], cosw_d.ap()[:, tok1])
                        nc.sync.dma_start(sinp_s[:, tok1], sinp_d.ap()[:, tok1])
                    emit_rope(b, c)
                    sq_out = emit_stats(b, c, mss)
                    if c == 0:
                        emit_scale(b, 0, sq_out[0], sq_out[1])
                nc.scalar.activation(sqv123[:], mss[:],
                                     mybir.ActivationFunctionType.Sqrt,
                                     bias=epsb_s[:], scale=8.0)
                for c in range(1, NCHB):
                    emit_scale(b, c, sqv123[:, 2 * (c - 1), :],
                               sqv123[:, 2 * (c - 1) + 1, :])

            def phase_C(b, qc):
                """Attention for batch b, q chunk qc (QC tokens, both heads)."""
                qs_ = slice(b * S + qc * QC, b * S + (qc + 1) * QC)
                oh0 = ohps.tile([128, NQT, 128], f32, tag="oh0")
                oh1 = ohps.tile([128, NQT, 128], f32, tag="oh1")
                oh = [oh0, oh1]
                for kt in range(KT):
                    ks = slice(b * S + kt * 128, b * S + (kt + 1) * 128)
                    sc = scps.tile([128, 2, QC], f32, tag="sc")
                    nc.tensor.matmul(sc[:, 0, :], khat[0:64, ks], qhat[0:64, qs_],
                                     start=True, stop=True)
                    nc.tensor.matmul(sc[:, 1, :], khat[64:128, ks], qhat[64:128, qs_],
                                     start=True, stop=True)
                    ex = expp.tile([128, 2, QC], EXDT, tag="ex")
                    nc.scalar.activation(ex[:], sc[:],
                                         mybir.ActivationFunctionType.Exp)
                    if DBG and b == 0 and qc == 0 and kt == 0:
                        nc.sync.dma_start(dbg["dbg_ex"].ap(),
                                          ex[:].rearrange("p a b -> p (a b)"))
                    st_once = os.environ.get("KPSTART", "once") == "once"
                    for h in range(2):
                        for qt in range(NQT):
                            nc.tensor.matmul(
                                oh[h][:, qt, 0:65],
                                ex[:, h, bass.ts(qt, 128)],
                                vtok[:, b, kt, h * 65:h * 65 + 65],
                                start=(kt == 0 and (qt == 0 or not st_once)),
                                stop=(kt == KT - 1 and qt == NQT - 1),
                                skip_group_check=True)
                # normalize (per-partition denominators) + transpose + osh DMA
                if DBG and b == 0 and qc == 0:
                    oh0c = nrm.tile([128, NQT, 65], f32, tag="oh0c")
                    nc.vector.tensor_copy(oh0c[:], oh[0][:, :, 0:65])
                    nc.sync.dma_start(dbg["dbg_oh0"].ap()[:, 0:NQT * 65],
                                      oh0c[:].rearrange("p a b -> p (a b)"))
                tr = psA.tile([64, 8, 128], f16, tag="x0")
                for h in range(2):
                    rec = nrm.tile([128, NQT], f32, tag="rec")
                    nc.vector.reciprocal_approx_fast(rec[:], oh[h][:, :, 64])
                    of = nrm.tile([128, NQT, 64], f16, tag=f"of{h}")
                    nc.vector.tensor_mul(of[:], oh[h][:, :, 0:64],
                                         rec[:].broadcast_to([128, NQT, 64]))
                    for qt in range(NQT):
                        nc.tensor.transpose(tr[:, h * NQT + qt, :], of[:, qt, :],
                                            iden_s[:])
                trs = nrm.tile([64, 8, 128], f16, tag="trs")
                nc.vector.tensor_copy(trs[:], tr[:])
                for qt in range(NQT):
                    blk = 2 * qc + qt // 2
                    col = (qt % 2) * 128
                    for h in range(2):
                        nc.sync.dma_start(
                            osh_d[b].ap()[blk * DL + h * 64: blk * DL + h * 64 + 64,
                                          col:col + 128],
                            trs[:, h * NQT + qt, :])

            def phase_A2A(b):
                nc.gpsimd.collective_compute(
                    "AllToAll", mybir.AluOpType.bypass,
                    replica_groups=[list(range(NC))],
                    ins=[osh_d[b].ap()], outs=[oga_d[b].ap()],
                )

            def phase_D(b):
                if b == 0:
                    nc.sync.dma_start(wo_s[:], wo_p)
                nc.sync.dma_start(g_s[:, b], oga_p[b])
                for do in range(8):
                    po = psA.tile([128, TBB], f32, tag="x0")
                    for dt_i in range(8):
                        nc.tensor.matmul(po[:], wo_s[:, dt_i, bass.ts(do, 128)],
                                         g_s[:, b, dt_i, :],
                                         start=(dt_i == 0), stop=(dt_i == 7))
                    ob = nrm.tile([128, TBB], f32, tag="ob")
                    nc.vector.tensor_copy(ob[:], po[:])
                    nc.sync.dma_start(out_d[b].ap()[bass.ts(do, 128), :], ob[:])

            # ---------------- emission schedule
            phase_AB(0)
            if DBG:
                nc.sync.dma_start(dbg["dbg_q01"].ap(), q01[:, 0:S])
                nc.sync.dma_start(dbg["dbg_qhat"].ap(), qhat[:, 0:S])
                nc.sync.dma_start(dbg["dbg_khat"].ap(), khat[:, 0:S])
                nc.sync.dma_start(dbg["dbg_vtok"].ap()[:, 0:KT * 130],
                                  vtok[:, 0].rearrange("p b c -> p (b c)"))
            phase_C(0, 0)
            phase_C(0, 1)
            phase_AB(1)
            phase_C(0, 2)
            phase_C(0, 3)
            phase_A2A(0)
            if DBG:
                nc.sync.dma_start(dbg["dbg_osh0"].ap(), osh_d[0].ap())
                nc.sync.dma_start(dbg["dbg_oga0"].ap(), oga_d[0].ap())
            phase_C(1, 0)
            phase_C(1, 1)
            phase_D(0)
            phase_C(1, 2)
            phase_C(1, 3)
            phase_A2A(1)
            phase_D(1)

    nc.compile()
    return nc


def make_inputs(x, position, w_qkv, w_out, norm_w, s=None):
    """Build the 8 per-core input dicts from full inputs."""
    assert (s or x.shape[1]) == S
    xT = np.ascontiguousarray(x.reshape(T, D).T).astype(np.float16)
    cos = position[0]   # [S, E]
    sin = position[1]
    nw = np.asarray(norm_w, np.float32)

    # rope permutation g: dest e<32 <- src 2e+1 (sign -1); dest 32+e <- src 2e
    g_idx = np.zeros(64, np.int64)
    sign = np.zeros(64, np.float32)
    for i in range(32):
        g_idx[i] = 2 * i + 1
        sign[i] = -1.0
        g_idx[32 + i] = 2 * i
        sign[32 + i] = 1.0

    # dest_e = qs_e*cosw_e + sign_e*m2_{g(e)} with m2_x = qs_x*sinp_x, so the
    # sin table lives in SOURCE index space: sinp_x = w_x * sin_{g^-1(x)}
    ginv = np.argsort(g_idx)
    cosw1 = (nw[None, :] * cos).T                      # [E, S]
    sinp1 = (sin[:, ginv] * nw[None, :]).T             # [E, S]
    cosw = np.tile(np.concatenate([cosw1, cosw1], 0), (1, B)).astype(np.float16)
    sinp = np.tile(np.concatenate([sinp1, sinp1], 0), (1, B)).astype(np.float16)

    # signed permutation matrix P: yp_e = sign_e * qs_{g(e)}
    P = np.zeros((64, 64), np.float32)
    for e in range(64):
        P[e, g_idx[e]] = sign[e]
    Pb = np.zeros((128, 128), np.float32)
    Pb[0:64, 0:64] = P
    Pb[64:128, 64:128] = P
    permT = np.ascontiguousarray(Pb.T).astype(np.float16)
    iden = np.eye(128, dtype=np.float16)

    woT = np.ascontiguousarray(np.asarray(w_out, np.float32).T).astype(np.float16)

    sel = np.zeros((128, 2), np.float16)
    sel[0:64, 0] = 1.0 / 64.0
    sel[64:128, 1] = 1.0 / 64.0
    wsel = np.zeros((2, 128), np.float32)
    wsel[0, 0:64] = 1.0
    wsel[1, 64:128] = 1.0

    w3 = np.asarray(w_qkv, np.float32).reshape(H, 3, E, D)
    in_maps = []
    for c in range(NC):
        h0, h1 = HPC * c, HPC * c + 1
        wqT = np.ascontiguousarray(
            np.concatenate([w3[h0, 0], w3[h1, 0]], 0).T).astype(np.float16)
        wkT = np.ascontiguousarray(
            np.concatenate([w3[h0, 1], w3[h1, 1]], 0).T).astype(np.float16)
        wvT = np.ascontiguousarray(
            np.concatenate([w3[h0, 2], w3[h1, 2]], 0).T).astype(np.float16)
        in_maps.append({
            "xT": xT, "wqT": wqT, "wkT": wkT, "wvT": wvT, "woT": woT,
            "cosw": cosw, "sinp": sinp, "permT": permT, "iden": iden,
            "sel": sel, "wsel": wsel,
        })
    return in_maps


def assemble(results, s=None):
    out = np.empty((B, S, D), np.float32)
    for c in range(NC):
        for b in range(B):
            out[b, c * TBB:(c + 1) * TBB, :] = results[c][f"out{b}"].T
    return out


_NC_CACHE = {}


def kernel(x, position, w_qkv, w_out, norm_w, heads):
    x = np.asarray(x, np.float32)
    position = np.asarray(position, np.float32)
    w_qkv = np.asarray(w_qkv, np.float32)
    w_out = np.asarray(w_out, np.float32)
    norm_w = np.asarray(norm_w, np.float32)
    s = x.shape[1]
    if s not in _NC_CACHE:
        _NC_CACHE[s] = build_nc(s)
    nc = _NC_CACHE[s]
    in_maps = make_inputs(x, position, w_qkv, w_out, norm_w, s=s)
    res = run_bass_kernel_spmd(nc, in_maps, list(range(NC)))
    return assemble(res.results, s=s)
